# revision 1
# baseline (speedup 1.0000x reference)
"""Trainium2 Bass kernel for the Roost-style GNN (nn_DescriptorNetworkTorch).

Data-parallel over graphs: 256 graphs of 16 fully-connected atoms are sharded
as 32 graphs per NeuronCore across 8 cores.  Each core runs the full
3-layer x 3-head message passing + crystal attention pooling on its shard;
no collectives are needed since every graph's nodes/edges are core-local.

Device-side layout highlights
  - node features live feature-major: fea_dup [128, 512] with rows 0:64 the
    F=64 features and rows 64:128 a duplicate (so self/nbr halves of the
    2F-contraction can be row-tiled onto partitions 0:64 / 64:128).
  - edges are the full 16x16 pair grid per graph (8192/core incl. the i==j
    diagonal); pair features are never materialized - the W1 matmuls read
    fea with stride-0 broadcast access patterns and the diagonal is removed
    by an additive -1e30 mask folded into the gate via a K=2 "rider" matmul
    (which also injects pow * ln(w_nbr) so softmax needs no extra pass).
  - selu(y) = lam*alpha*(min(e^y,1)-1) + lam*relu(y) is computed as one ACT
    exp pass + a cheap min + a relu pass, and the two branches are contracted
    by the W2 matmuls with stacked K=512 weights [lam*alpha*W2; lam*W2]; all
    additive constants (b2, -lam*alpha*sum(W2), head-mean 1/3) are folded
    into rider matmuls / host-packed weights.
  - softmax is computed on head-paired [128, 512] tiles (head0 rows 0:64,
    head1 rows 64:128) with the gate replicated across partitions by
    replicated-column W2 weights, so the attention weights are already
    broadcast for the msg apply and no cross-partition moves are needed.
"""

import numpy as np

G, K, F, EMB, HID, L, H = 256, 16, 64, 200, 256, 3, 3
NCORES = 8
GPC = G // NCORES          # graphs per core
N = GPC * K                # nodes per core (512)
E = GPC * K * K            # all-pair edges per core (8192)
NEG = E // 512             # number of 512-edge groups (16)
LAM = 1.0507009873554804934193349852946
ALPHA = 1.6732632423543772848170429916717
MASKNEG = -1e30

_PROGRAM_CACHE = {}


def _build_program():
    import concourse.bass as bass
    import concourse.bacc as bacc
    import concourse.mybir as mybir
    import concourse.tile as tile

    dt = mybir.dt
    AF = mybir.ActivationFunctionType
    ALU = mybir.AluOpType
    AX = mybir.AxisListType
    f32 = dt.float32
    bf16 = dt.bfloat16

    nc = bacc.Bacc("TRN2", target_bir_lowering=False, debug=False,
                   num_devices=NCORES)

    # ---------------- DRAM I/O ----------------
    d_eft = nc.dram_tensor("eft", [EMB, N], bf16, kind="ExternalInput")
    d_wi = nc.dram_tensor("wipack", [128, 126], bf16, kind="ExternalInput")
    d_binit = nc.dram_tensor("binit", [63, 1], f32, kind="ExternalInput")
    d_wrow = nc.dram_tensor("wrow", [1, N], f32, kind="ExternalInput")
    d_wlogE = nc.dram_tensor("wlogE", [1, E], f32, kind="ExternalInput")
    d_maskE = nc.dram_tensor("maskE", [1, E], bf16, kind="ExternalInput")
    d_maskC = nc.dram_tensor("maskC", [1, N], bf16, kind="ExternalInput")
    d_ones = nc.dram_tensor("ones1", [1, 512], bf16, kind="ExternalInput")
    d_wlogC = nc.dram_tensor("wlogC", [1, N], f32, kind="ExternalInput")
    d_w1, d_b1, d_wms, d_wgs, d_gr, d_mc = [], [], [], [], [], []
    for l in range(L):
        d_w1.append(nc.dram_tensor(f"w1pack{l}", [128, 1536], bf16, kind="ExternalInput"))
        d_b1.append(nc.dram_tensor(f"b1pack{l}", [128, 12], f32, kind="ExternalInput"))
        d_wms.append(nc.dram_tensor(f"wms{l}", [128, 768], bf16, kind="ExternalInput"))
        d_wgs.append(nc.dram_tensor(f"wgs{l}", [128, 768], bf16, kind="ExternalInput"))
        d_gr.append(nc.dram_tensor(f"grl{l}", [4, 256], bf16, kind="ExternalInput"))
        d_mc.append(nc.dram_tensor(f"mcl{l}", [1, 192], bf16, kind="ExternalInput"))
    d_cw1 = nc.dram_tensor("cw1pack", [128, 1536], bf16, kind="ExternalInput")
    d_cb1 = nc.dram_tensor("cb1pack", [128, 12], f32, kind="ExternalInput")
    d_cwms = nc.dram_tensor("cwms", [128, 768], bf16, kind="ExternalInput")
    d_cwgs = nc.dram_tensor("cwgs", [128, 768], bf16, kind="ExternalInput")
    d_cgr = nc.dram_tensor("cgr", [4, 256], bf16, kind="ExternalInput")
    d_cmc = nc.dram_tensor("cmc", [1, 192], bf16, kind="ExternalInput")
    d_out = nc.dram_tensor("out", [F, GPC], f32, kind="ExternalOutput")

    with tile.TileContext(nc) as tc:
        with tc.tile_pool(name="const", bufs=1) as cp, \
             tc.tile_pool(name="fea", bufs=2) as fp, \
             tc.tile_pool(name="hid", bufs=13) as hp, \
             tc.tile_pool(name="zp", bufs=3) as zp, \
             tc.tile_pool(name="node", bufs=2) as np_, \
             tc.tile_pool(name="pre_ps", bufs=4, space="PSUM") as pps, \
             tc.tile_pool(name="w2_ps", bufs=4, space="PSUM") as wps:

            # ---- load constants ----
            ef1 = cp.tile([128, N], bf16, tag="ef1")
            ef2 = cp.tile([72, N], bf16, tag="ef2")
            nc.sync.dma_start(ef1[:], d_eft[0:128, :])
            nc.sync.dma_start(ef2[:], d_eft[128:200, :])
            wi = cp.tile([128, 126], bf16, tag="wi")
            nc.sync.dma_start(wi[:], d_wi[:])
            binit = cp.tile([63, 1], f32, tag="binit")
            nc.sync.dma_start(binit[:], d_binit[:])
            wrow = cp.tile([1, N], f32, tag="wrow")
            nc.sync.dma_start(wrow[:], d_wrow[:])
            wlogE = cp.tile([1, E], f32, tag="wlogE")
            nc.sync.dma_start(wlogE[:], d_wlogE[:])

            ones1 = cp.tile([1, 512], bf16, tag="ones1")
            nc.sync.dma_start(ones1[:], d_ones[:])
            wlogC = cp.tile([1, N], f32, tag="wlogC")
            nc.sync.dma_start(wlogC[:], d_wlogC[:])
            w1s, b1s, wmss, wgss, grs, mcs = [], [], [], [], [], []
            for l in range(L):
                t = cp.tile([128, 1536], bf16, tag=f"w1_{l}")
                nc.sync.dma_start(t[:], d_w1[l][:]); w1s.append(t)
                t = cp.tile([128, 12], f32, tag=f"b1_{l}")
                nc.sync.dma_start(t[:], d_b1[l][:]); b1s.append(t)
                t = cp.tile([128, 768], bf16, tag=f"wms_{l}")
                nc.sync.dma_start(t[:], d_wms[l][:]); wmss.append(t)
                t = cp.tile([128, 768], bf16, tag=f"wgs_{l}")
                nc.sync.dma_start(t[:], d_wgs[l][:]); wgss.append(t)
                t = cp.tile([4, 256], bf16, tag=f"gr_{l}")
                nc.sync.dma_start(t[:], d_gr[l][:]); grs.append(t)
                t = cp.tile([1, 192], bf16, tag=f"mc_{l}")
                nc.sync.dma_start(t[:], d_mc[l][:]); mcs.append(t)
            cw1 = cp.tile([128, 1536], bf16, tag="cw1")
            nc.sync.dma_start(cw1[:], d_cw1[:])
            cb1 = cp.tile([128, 12], f32, tag="cb1")
            nc.sync.dma_start(cb1[:], d_cb1[:])
            cwms = cp.tile([128, 768], bf16, tag="cwms")
            nc.sync.dma_start(cwms[:], d_cwms[:])
            cwgs = cp.tile([128, 768], bf16, tag="cwgs")
            nc.sync.dma_start(cwgs[:], d_cwgs[:])
            cgr = cp.tile([4, 256], bf16, tag="cgr")
            nc.sync.dma_start(cgr[:], d_cgr[:])
            cmc = cp.tile([1, 192], bf16, tag="cmc")
            nc.sync.dma_start(cmc[:], d_cmc[:])

            # ln(w_nbr) / ln(w_node), then split into bf16 hi/lo rider rows
            nc.scalar.activation(wlogE[:], wlogE[:], AF.Ln)
            nc.scalar.activation(wlogC[:], wlogC[:], AF.Ln)
            riderE3 = cp.tile([4, E], bf16, tag="riderE3")
            hiE = cp.tile([1, E], bf16, tag="hiE")
            loE = cp.tile([1, E], bf16, tag="loE")
            nc.vector.tensor_copy(hiE[:], wlogE[:])
            nc.vector.tensor_tensor(out=wlogE[:], in0=wlogE[:],
                                    in1=hiE[:], op=ALU.subtract)
            nc.vector.tensor_copy(loE[:], wlogE[:])
            nc.sync.dma_start(riderE3[0:1, :], hiE[:])
            nc.sync.dma_start(riderE3[1:2, :], loE[:])
            nc.sync.dma_start(riderE3[2:3, :], hiE[:])
            nc.sync.dma_start(riderE3[3:4, :], d_maskE[:])
            riderC3 = cp.tile([4, N], bf16, tag="riderC3")
            hiC = cp.tile([1, N], bf16, tag="hiC")
            loC = cp.tile([1, N], bf16, tag="loC")
            nc.vector.tensor_copy(hiC[:], wlogC[:])
            nc.vector.tensor_tensor(out=wlogC[:], in0=wlogC[:],
                                    in1=hiC[:], op=ALU.subtract)
            nc.vector.tensor_copy(loC[:], wlogC[:])
            nc.sync.dma_start(riderC3[0:1, :], hiC[:])
            nc.sync.dma_start(riderC3[1:2, :], loC[:])
            nc.sync.dma_start(riderC3[2:3, :], hiC[:])
            nc.sync.dma_start(riderC3[3:4, :], d_maskC[:])

            # ---- initial embed ----
            fea = fp.tile([128, N], f32, tag="fea")
            for c in range(N // 512):
                sl = slice(c * 512, (c + 1) * 512)
                emb_ps = wps.tile([63, 512], f32, tag="w2ps")
                nc.tensor.matmul(emb_ps[:], (wi[0:128, 0:63]), (ef1[:, sl]),
                                 start=True, stop=False)
                nc.tensor.matmul(emb_ps[:], (wi[0:72, 63:126]), (ef2[:, sl]),
                                 start=False, stop=True)
                nc.scalar.activation(fea[0:63, sl], emb_ps[:], AF.Identity,
                                     bias=binit[:], scale=1.0)
            nc.sync.dma_start(fea[63:64, :], wrow[:])
            nc.sync.dma_start(fea[64:128, :], fea[0:64, :])

            ONE = 1.0

            def mlp_hidden(l, h, w1t, b1t, pair, relu_on_act):
                """pre-act + selu decomposition for one (l,h,mlp,half) family.
                Returns dict (mlp, half, kind) -> [128,512] sbuf tile."""
                out = {}
                for mlp in range(2):
                    for half in range(2):
                        wcol = ((h * 2 + mlp) * 2 + half) * 128
                        bcol = (h * 2 + mlp) * 2 + half
                        pre = pps.tile([128, 512], f32, tag="pre")
                        nc.tensor.matmul(pre[:], (w1t[:, wcol:wcol + 128]), (pair[:]),
                                 start=True, stop=True)
                        v = hp.tile([128, 512], bf16, tag="hv")
                        nc.scalar.activation(v[:], pre[:], AF.Exp,
                                             bias=b1t[:, bcol:bcol + 1], scale=1.0)
                        nc.vector.tensor_scalar(v[:], v[:], ONE, None,
                                                op0=ALU.min)
                        w = hp.tile([128, 512], bf16, tag="hw")
                        if relu_on_act:
                            nc.scalar.activation(w[:], pre[:], AF.Relu,
                                                 bias=b1t[:, bcol:bcol + 1],
                                                 scale=1.0)
                        else:
                            nc.vector.tensor_scalar(w[:], pre[:],
                                                    b1t[:, bcol:bcol + 1], 0.0,
                                                    op0=ALU.add, op1=ALU.max)
                        out[(mlp, half)] = (v, w)
                return out

            def w2_stage(hid, wgst, wmst, grt, mct, rr_ones, rr_wm3):
                """stacked-K W2 matmuls for a head pair + solo head.
                hid: {h: {(mlp,half): (v,w)}}
                returns (gate01_ps[128,512], msg01_ps[128,512],
                         gate2_ps[128,512], msg2_ps[64,512])"""
                def rhs_chunk(h, mlp, kc):
                    v, w = hid[h][(mlp, kc % 2)]
                    return (v if kc < 2 else w)[:]

                gps = wps.tile([128, 512], f32, tag="w2ps")
                nc.tensor.matmul(gps[:], (grt[0:4, 0:128]), (rr_wm3),
                                 start=True, stop=False, skip_group_check=True)
                for kc in range(4):
                    nc.tensor.matmul(gps[0:64, :], (wgst[:, 0 * 256 + kc * 64: 0 * 256 + kc * 64 + 64]), (rhs_chunk(0, 0, kc)),
                                 start=False, stop=False, skip_group_check=True)
                for kc in range(4):
                    nc.tensor.matmul(gps[64:128, :], (wgst[:, 1 * 256 + kc * 64: 1 * 256 + kc * 64 + 64]), (rhs_chunk(1, 0, kc)),
                                 start=False, stop=(kc == 3),
                                     tile_position=(0, 64), skip_group_check=True)
                mps = wps.tile([128, 512], f32, tag="w2ps")
                nc.tensor.matmul(mps[:], (mct[:, 0:128]), (rr_ones),
                                 start=True, stop=False, skip_group_check=True)
                for kc in range(4):
                    nc.tensor.matmul(mps[0:64, :], (wmst[:, 0 * 256 + kc * 64: 0 * 256 + kc * 64 + 64]), (rhs_chunk(0, 1, kc)),
                                 start=False, stop=False, skip_group_check=True)
                for kc in range(4):
                    nc.tensor.matmul(mps[64:128, :], (wmst[:, 1 * 256 + kc * 64: 1 * 256 + kc * 64 + 64]), (rhs_chunk(1, 1, kc)),
                                 start=False, stop=(kc == 3),
                                     tile_position=(0, 64), skip_group_check=True)
                g2ps = wps.tile([128, 512], f32, tag="w2ps")
                nc.tensor.matmul(g2ps[:], (grt[0:4, 128:256]), (rr_wm3),
                                 start=True, stop=False, skip_group_check=True)
                for kc in range(4):
                    nc.tensor.matmul(g2ps[0:64, :], (wgst[:, 2 * 256 + kc * 64: 2 * 256 + kc * 64 + 64]), (rhs_chunk(2, 0, kc)),
                                 start=False, stop=False, skip_group_check=True)
                for kc in range(4):
                    nc.tensor.matmul(g2ps[64:128, :], (wgst[:, 2 * 256 + kc * 64: 2 * 256 + kc * 64 + 64]), (rhs_chunk(2, 0, kc)),
                                 start=False, stop=(kc == 3),
                                     tile_position=(0, 64), skip_group_check=True)
                m2ps = wps.tile([64, 512], f32, tag="w2ps")
                nc.tensor.matmul(m2ps[:], (mct[:, 128:192]), (rr_ones),
                                 start=True, stop=False, skip_group_check=True)
                for kc in range(4):
                    nc.tensor.matmul(m2ps[:], (wmst[:, 2 * 256 + kc * 64: 2 * 256 + kc * 64 + 64]), (rhs_chunk(2, 1, kc)),
                                 start=False, stop=(kc == 3), skip_group_check=True)
                return gps, mps, g2ps, m2ps

            def softmax_apply(gps, mps, g2ps, m2ps, dn01, rn01, dn2, rn2, seg):
                z = zp.tile([128, 512], f32, tag="z")
                nc.scalar.activation(z[:], gps[:], AF.Exp)
                nc.vector.tensor_reduce(
                    out=dn01[:, seg], in_=z[:].rearrange("p (s j) -> p s j", j=K),
                    axis=AX.X, op=ALU.add)
                prod = zp.tile([128, 512], f32, tag="prod")
                nc.vector.tensor_tensor(out=prod[:], in0=mps[:], in1=z[:],
                                        op=ALU.mult)
                nc.vector.tensor_reduce(
                    out=rn01[:, seg], in_=prod[:].rearrange("p (s j) -> p s j", j=K),
                    axis=AX.X, op=ALU.add)
                z2 = zp.tile([128, 512], f32, tag="z2")
                nc.scalar.activation(z2[:], g2ps[:], AF.Exp)
                nc.vector.tensor_reduce(
                    out=dn2[:, seg], in_=z2[0:64, :].rearrange("p (s j) -> p s j", j=K),
                    axis=AX.X, op=ALU.add)
                prod2 = zp.tile([64, 512], f32, tag="prod2")
                nc.vector.tensor_tensor(out=prod2[:], in0=m2ps[:], in1=z2[0:64, :],
                                        op=ALU.mult)
                nc.vector.tensor_reduce(
                    out=rn2[:, seg], in_=prod2[:].rearrange("p (s j) -> p s j", j=K),
                    axis=AX.X, op=ALU.add)

            def finish_update(dn01, rn01, dn2, rn2, nseg):
                nc.vector.tensor_scalar(dn01[:], dn01[:], 1e-10, None, op0=ALU.add)
                nc.vector.tensor_scalar(dn2[:], dn2[:], 1e-10, None, op0=ALU.add)
                nc.vector.reciprocal(dn01[:], dn01[:])
                nc.vector.reciprocal(dn2[:], dn2[:])
                nc.vector.tensor_tensor(out=rn01[:], in0=rn01[:], in1=dn01[:],
                                        op=ALU.mult)
                nc.vector.tensor_tensor(out=rn2[:], in0=rn2[:], in1=dn2[:],
                                        op=ALU.mult)
                # cross-partition: bring head1 rows down to partitions 0:64
                upd1lo = np_.tile([64, nseg], f32, tag="upd1lo")
                nc.sync.dma_start(upd1lo[:], rn01[64:128, :])
                nc.vector.tensor_tensor(out=rn2[:], in0=rn2[:], in1=upd1lo[:],
                                        op=ALU.add)
                nc.vector.tensor_tensor(out=rn2[:], in0=rn2[:], in1=rn01[0:64, :],
                                        op=ALU.add)
                return rn2

            # ---------------- message passing layers ----------------
            for l in range(L):
                dn01 = np_.tile([128, N], f32, tag="dn01")
                rn01 = np_.tile([128, N], f32, tag="rn01")
                dn2 = np_.tile([64, N], f32, tag="dn2")
                rn2 = np_.tile([64, N], f32, tag="rn2")
                for eg in range(NEG):
                    col0 = eg * 2 * K           # first node column of the 2 graphs
                    self_src = (fea[0:64, col0:col0 + 32]
                                .rearrange("p (g i) -> p g i", g=2)
                                .unsqueeze(3).broadcast_to([64, 2, K, K]))
                    nbr_src = (fea[64:128, col0:col0 + 32]
                               .rearrange("p (g j) -> p g j", g=2)
                               .unsqueeze(2).broadcast_to([64, 2, K, K]))
                    pair = hp.tile([128, 512], bf16, tag="pair", bufs=5)
                    nc.vector.tensor_copy(
                        pair[0:64, :].rearrange("p (g i j) -> p g i j", g=2, i=K),
                        self_src)
                    nc.vector.tensor_copy(
                        pair[64:128, :].rearrange("p (g i j) -> p g i j", g=2, i=K),
                        nbr_src)
                    esl = slice(eg * 512, (eg + 1) * 512)
                    hid = {}
                    for h in range(H):
                        hid[h] = mlp_hidden(l, h, w1s[l], b1s[l], pair,
                                            relu_on_act=(h == 0))
                    # hid dict keyed (mlp, half); mlp 0 = gate, 1 = msg
                    gps, mps, g2ps, m2ps = w2_stage(
                        hid, wgss[l], wmss[l], grs[l], mcs[l],
                        ones1[:], riderE3[:, esl])
                    seg = slice(eg * 32, (eg + 1) * 32)
                    softmax_apply(gps, mps, g2ps, m2ps, dn01, rn01, dn2, rn2, seg)
                upd = finish_update(dn01, rn01, dn2, rn2, N)
                fea2 = fp.tile([128, N], f32, tag="fea")
                nc.vector.tensor_tensor(out=fea2[0:64, :], in0=fea[0:64, :],
                                        in1=upd[:], op=ALU.add)
                nc.sync.dma_start(fea2[64:128, :], fea2[0:64, :])
                fea = fea2

            # ---------------- crystal pooling ----------------
            dn01 = np_.tile([128, GPC], f32, tag="dn01")
            rn01 = np_.tile([128, GPC], f32, tag="rn01")
            dn2 = np_.tile([64, GPC], f32, tag="dn2")
            rn2 = np_.tile([64, GPC], f32, tag="rn2")
            fea_bf = fp.tile([64, N], bf16, tag="fea_bf")
            nc.vector.tensor_copy(fea_bf[:], fea[0:64, :])
            for eg in range(N // 512):
                esl = slice(eg * 512, (eg + 1) * 512)
                hid = {}
                for h in range(H):
                    out = {}
                    for mlp in range(2):
                        for half in range(2):
                            wcol = ((h * 2 + mlp) * 2 + half) * 128
                            bcol = (h * 2 + mlp) * 2 + half
                            pre = pps.tile([128, 512], f32, tag="pre")
                            nc.tensor.matmul(pre[:], (cw1[0:64, wcol:wcol + 128]), (fea_bf[:, esl]),
                                 start=True, stop=True)
                            v = hp.tile([128, 512], bf16, tag="hv")
                            nc.scalar.activation(v[:], pre[:], AF.Exp,
                                                 bias=cb1[:, bcol:bcol + 1], scale=1.0)
                            nc.vector.tensor_scalar(v[:], v[:], ONE, None,
                                                    op0=ALU.min)
                            w = hp.tile([128, 512], bf16, tag="hw")
                            if h == 0:
                                nc.scalar.activation(w[:], pre[:], AF.Relu,
                                                     bias=cb1[:, bcol:bcol + 1],
                                                     scale=1.0)
                            else:
                                nc.vector.tensor_scalar(w[:], pre[:],
                                                        cb1[:, bcol:bcol + 1], 0.0,
                                                        op0=ALU.add, op1=ALU.max)
                            out[(mlp, half)] = (v, w)
                    hid[h] = out
                gps, mps, g2ps, m2ps = w2_stage(
                    hid, cwgs, cwms, cgr, cmc,
                    ones1[:], riderC3[:, esl])
                seg = slice(eg * 32, (eg + 1) * 32)
                softmax_apply(gps, mps, g2ps, m2ps, dn01, rn01, dn2, rn2, seg)
            cry = finish_update(dn01, rn01, dn2, rn2, GPC)
            nc.sync.dma_start(d_out[:], cry[:])

    nc.compile()
    return nc


def _prep_core_inputs(core, elem_weights, elem_fea_in, W_init, b_init,
                      mg_W1, mg_b1, mg_W2, mg_b2, mm_W1, mm_b1, mm_W2, mm_b2,
                      m_pow, cg_W1, cg_b1, cg_W2, cg_b2, cm_W1, cm_b1, cm_W2,
                      cm_b2, c_pow):
    import ml_dtypes
    f = np.float32
    bf = ml_dtypes.bfloat16
    n0 = core * N
    w = np.ascontiguousarray(elem_weights[n0:n0 + N]).astype(f)
    ef = np.ascontiguousarray(elem_fea_in[n0:n0 + N]).astype(f)

    ins = {}
    ins["eft"] = np.ascontiguousarray(ef.T).astype(bf)
    wi = np.zeros((128, 126), f)
    wi[0:128, 0:63] = W_init[0:128]
    wi[0:72, 63:126] = W_init[128:200]
    ins["wipack"] = wi.astype(bf)
    ins["binit"] = b_init.reshape(63, 1).astype(f)
    ins["wrow"] = w.reshape(1, N)

    # edge rider rows: ones / w[nbr] (-> ln on device) / additive diag mask
    j_of_e = np.tile(np.arange(K), GPC * K)                       # nbr j per edge
    gi_of_e = np.repeat(np.arange(GPC * K), K)                    # seg per edge
    g_of_e = gi_of_e // K
    i_of_e = gi_of_e % K
    wn = w[g_of_e * K + j_of_e]
    ins["wlogE"] = wn.reshape(1, E)
    ins["maskE"] = np.where(i_of_e == j_of_e, MASKNEG, 0.0).astype(bf).reshape(1, E)
    ins["maskC"] = np.zeros((1, N), bf)
    ins["ones1"] = np.ones((1, 512), bf)
    ins["wlogC"] = w.reshape(1, N)

    def pack_layer(W1g, b1g, W2g, W1m, b1m, W2m, b2m, pw):
        # W1g/W1m: [H,2F,HID]; W2g: [H,HID]; W2m: [H,HID,F]; b2m: [H,F]; pw [H]
        w1 = np.zeros((128, 1536), f)
        b1 = np.zeros((128, 12), f)
        wms = np.zeros((128, 768), f)
        wgs = np.zeros((128, 768), f)
        gr = np.zeros((4, 256), f)
        mc = np.zeros((1, 192), f)
        for h in range(H):
            for mlp, (W1x, b1x) in enumerate(((W1g[h], b1g[h]), (W1m[h], b1m[h]))):
                for half in range(2):
                    w1[:, ((h * 2 + mlp) * 2 + half) * 128:
                         ((h * 2 + mlp) * 2 + half) * 128 + 128] = \
                        W1x[:, half * 128:(half + 1) * 128]
                    b1[:, (h * 2 + mlp) * 2 + half] = b1x[half * 128:(half + 1) * 128]
            mstack = np.concatenate([LAM * ALPHA / H * W2m[h],
                                     LAM / H * W2m[h]], axis=0)      # [512, F]
            gstack = np.concatenate([LAM * ALPHA * W2g[h],
                                     LAM * W2g[h]], axis=0)          # [512]
            gstack = np.repeat(gstack[:, None], 64, axis=1)          # [512, 64]
            for kc in range(4):
                wms[:, h * 256 + kc * 64: h * 256 + kc * 64 + 64] = \
                    mstack[kc * 128:(kc + 1) * 128]
                wgs[:, h * 256 + kc * 64: h * 256 + kc * 64 + 64] = \
                    gstack[kc * 128:(kc + 1) * 128]
            Ch = (b2m[h] - LAM * ALPHA * W2m[h].sum(axis=0)) / H     # [F]
            pw_hi = np.float32(bf(pw[h]))
            pw_lo = np.float32(pw[h]) - pw_hi
            cols = (slice(h * 64, (h + 1) * 64) if h < 2 else slice(128, 256))
            gr[0:2, cols] = pw_hi
            gr[2, cols] = pw_lo
            if h < 2:
                mc[0, h * 64:(h + 1) * 64] = Ch
            else:
                mc[0, 128:192] = Ch
        gr[3, :] = 1.0
        return w1, b1, wms, wgs, gr, mc

    for l in range(L):
        w1, b1, wms, wgs, gr, mc = pack_layer(
            mg_W1[l], mg_b1[l], mg_W2[l], mm_W1[l], mm_b1[l], mm_W2[l],
            mm_b2[l], m_pow[l])
        ins[f"w1pack{l}"] = w1.astype(bf)
        ins[f"b1pack{l}"] = b1
        ins[f"wms{l}"] = wms.astype(bf)
        ins[f"wgs{l}"] = wgs.astype(bf)
        ins[f"grl{l}"] = gr.astype(bf)
        ins[f"mcl{l}"] = mc.astype(bf)

    # crystal: input dim F=64 -> W1 slots are [64, HID]; embed into 2F rows 0:64
    cW1g = np.zeros((H, 128, HID), np.float32)
    cW1g[:, 0:F, :] = cg_W1
    cW1m = np.zeros((H, 128, HID), np.float32)
    cW1m[:, 0:F, :] = cm_W1
    w1, b1, wms, wgs, gr, mc = pack_layer(
        cW1g, cg_b1, cg_W2, cW1m, cm_b1, cm_W2, cm_b2, c_pow)
    ins["cw1pack"] = w1.astype(bf)
    ins["cb1pack"] = b1
    ins["cwms"] = wms.astype(bf)
    ins["cwgs"] = wgs.astype(bf)
    ins["cgr"] = gr.astype(bf)
    ins["cmc"] = mc.astype(bf)
    return {k: np.ascontiguousarray(v) for k, v in ins.items()}


def _check_structure(batch, self_idx, nbr_idx):
    exp_batch = np.repeat(np.arange(G, dtype=np.int64), K)
    i = np.arange(K)
    src, dst = np.meshgrid(i, i, indexing="ij")
    m = src != dst
    offs = (np.arange(G) * K)[:, None]
    exp_self = (offs + src[m][None, :]).reshape(-1)
    exp_nbr = (offs + dst[m][None, :]).reshape(-1)
    if not (np.array_equal(np.asarray(batch, np.int64), exp_batch)
            and np.array_equal(np.asarray(self_idx, np.int64), exp_self)
            and np.array_equal(np.asarray(nbr_idx, np.int64), exp_nbr)):
        raise NotImplementedError(
            "kernel specialized to the 256x16 fully-connected mesh structure")


def kernel(**inputs):
    from concourse.bass_utils import run_bass_kernel_spmd

    _check_structure(inputs["batch"], inputs["self_idx"], inputs["nbr_idx"])
    args = {k: np.asarray(v) for k, v in inputs.items()
            if k not in ("batch", "self_idx", "nbr_idx")}

    if "nc" not in _PROGRAM_CACHE:
        _PROGRAM_CACHE["nc"] = _build_program()
    nc = _PROGRAM_CACHE["nc"]

    in_maps = [_prep_core_inputs(c, **args) for c in range(NCORES)]
    res = run_bass_kernel_spmd(nc, in_maps, list(range(NCORES)))
    out = np.concatenate([res.results[c]["out"].T for c in range(NCORES)], axis=0)
    return out.astype(np.float32)

